# revision 2
# baseline (speedup 1.0000x reference)
"""Trainium2 Bass kernel for CPU-optimized Multi-Query Attention.

Full inputs in, full output out. Sharding: data-parallel over batch (2) x
sequence-parallel over queries (4) = 8 cores, no collectives. Each core:
  - projects its 512-query slice (q) plus the full K/V of its batch,
  - runs 16-head MQA attention in transposed layouts (scoresT = [keys, queries])
    so softmax sums ride as an extra ones-column in the ctx matmul,
  - o-projects and writes outT [1024, 512]; host transposes/concats.
All matmul operands are float32r (TF32) for full PE rate.
"""
import numpy as np

B, S, D, H, DK = 2, 2048, 1024, 16, 64
SQ = S // 4          # queries per core
NC = 8               # cores
SCALE = DK ** -0.5
NCH = S // 128       # 16 key chunks
NI = D // 128        # 8 contraction chunks for projections
NT = D // 128        # 8 output tiles

_CACHE = {}


def _build():
    import concourse.bacc as bacc
    import concourse.bass as bass
    import concourse.tile as tile
    from concourse import mybir

    F32 = mybir.dt.float32
    F32R = mybir.dt.float32r
    Exp = mybir.ActivationFunctionType.Exp

    nc = bacc.Bacc("TRN2", target_bir_lowering=False, debug=False, num_devices=NC)

    qT_d = nc.dram_tensor("qT", [D, SQ], F32R, kind="ExternalInput").ap()
    kT_d = nc.dram_tensor("kT", [D, S], F32R, kind="ExternalInput").ap()
    vT_d = nc.dram_tensor("vT", [D, S], F32R, kind="ExternalInput").ap()
    wq_d = nc.dram_tensor("WqTs", [D, D], F32R, kind="ExternalInput").ap()
    bq_d = nc.dram_tensor("bqs", [1, D], F32R, kind="ExternalInput").ap()
    wk_d = nc.dram_tensor("WkT", [D, DK], F32R, kind="ExternalInput").ap()
    bk_d = nc.dram_tensor("bk", [1, DK], F32R, kind="ExternalInput").ap()
    wv_d = nc.dram_tensor("WvT", [D, DK], F32R, kind="ExternalInput").ap()
    bv_d = nc.dram_tensor("bv", [1, DK], F32R, kind="ExternalInput").ap()
    wo_d = nc.dram_tensor("WoT", [D, D], F32R, kind="ExternalInput").ap()
    bo_d = nc.dram_tensor("bo", [1, D], F32R, kind="ExternalInput").ap()
    id_d = nc.dram_tensor("ident", [DK, DK], F32R, kind="ExternalInput").ap()
    ones_h = nc.dram_tensor("ones", [1, SQ], F32R, kind="ExternalInput")
    ones_d = ones_h.ap()
    out_d = nc.dram_tensor("outT", [D, SQ], F32, kind="ExternalOutput").ap()

    with tile.TileContext(nc) as tc:
        with (
            tc.tile_pool(name="single", bufs=1) as single,
            tc.tile_pool(name="wpool", bufs=8) as wpool,
            tc.tile_pool(name="stream", bufs=8) as stream,
            tc.tile_pool(name="qpool", bufs=8) as qpool,
            tc.tile_pool(name="expp", bufs=20) as expp,
            tc.tile_pool(name="small", bufs=2) as small,
            tc.tile_pool(name="ps_proj", bufs=2, space="PSUM") as ps_proj,
            tc.tile_pool(name="ps_sc", bufs=3, space="PSUM") as ps_sc,
            tc.tile_pool(name="ps_ctx", bufs=2, space="PSUM") as ps_ctx,
            tc.tile_pool(name="ps_bc", bufs=1, space="PSUM") as ps_bc,
        ):
            # ---- persistent SBUF ----
            khT = single.tile([128, S], F32R)          # kh^T duplicated halves
            vhT = single.tile([DK, S], F32R)
            vhA = single.tile([128, NCH, DK + 1], F32R)  # [vh | ones] per chunk
            qhT = single.tile([128, NT, SQ], F32R)
            ctxn = single.tile([128, NT, SQ], F32R)
            wk_sb = single.tile([128, NI, DK], F32R)
            wv_sb = single.tile([128, NI, DK], F32R)
            bk_sb = single.tile([1, DK], F32R)
            bv_sb = single.tile([1, DK], F32R)
            bq_sb = single.tile([1, D], F32R)
            bo_sb = single.tile([1, D], F32R)
            id_sb = single.tile([DK, DK], F32R)
            ones_row = single.tile([1, SQ], F32R)
            sel = single.tile([65, DK], F32R)

            # ---- constant / weight loads ----
            nc.sync.dma_start(out=ones_row, in_=ones_d)
            nc.sync.dma_start(out=sel[64:65, :], in_=ones_d[0:1, 0:DK])
            nc.sync.dma_start(out=id_sb, in_=id_d)
            nc.sync.dma_start(out=bk_sb, in_=bk_d)
            nc.sync.dma_start(out=bv_sb, in_=bv_d)
            nc.sync.dma_start(out=bq_sb, in_=bq_d)
            nc.sync.dma_start(out=bo_sb, in_=bo_d)
            nc.sync.dma_start(out=wk_sb, in_=wk_d.rearrange("(c p) d -> p c d", p=128))
            nc.sync.dma_start(out=wv_sb, in_=wv_d.rearrange("(c p) d -> p c d", p=128))
            nc.sync.dma_start(
                out=vhA[:, :, DK : DK + 1],
                in_=bass.AP(tensor=ones_h, offset=0, ap=[[0, 128], [0, NCH], [1, 1]]),
            )
            wq_sb = []
            for i in range(NI):
                w = wpool.tile([128, D], F32R, tag="bigw")
                nc.sync.dma_start(out=w, in_=wq_d[128 * i : 128 * (i + 1), :])
                wq_sb.append(w)

            # ---- K/V projections (full batch sequence) ----
            for j in range(4):
                js = slice(512 * j, 512 * (j + 1))
                kc, vc = [], []
                for i in range(NI):
                    t = stream.tile([128, 512], F32R, tag="stream")
                    nc.sync.dma_start(out=t, in_=kT_d[128 * i : 128 * (i + 1), js])
                    kc.append(t)
                for i in range(NI):
                    t = stream.tile([128, 512], F32R, tag="stream")
                    nc.sync.dma_start(out=t, in_=vT_d[128 * i : 128 * (i + 1), js])
                    vc.append(t)
                kps = ps_proj.tile([DK, 512], F32, tag="proj")
                for i in range(NI):
                    nc.tensor.matmul(kps, wk_sb[:, i, :], kc[i], start=(i == 0), stop=False)
                nc.tensor.matmul(kps, bk_sb, ones_row, start=False, stop=True)
                nc.vector.tensor_copy(out=khT[0:DK, js], in_=kps)
                vps = ps_proj.tile([DK, 512], F32, tag="proj")
                for i in range(NI):
                    nc.tensor.matmul(vps, wv_sb[:, i, :], vc[i], start=(i == 0), stop=False)
                nc.tensor.matmul(vps, bv_sb, ones_row, start=False, stop=True)
                nc.vector.tensor_copy(out=vhT[:, js], in_=vps)
                # duplicate khT rows into partitions 64..127 for row-packed matmuls
                nc.sync.dma_start(out=khT[64:128, js], in_=khT[0:DK, js])

            # vh = vhT^T via PE transpose, into [vh | ones] tiles
            for j in range(NCH):
                tr = ps_proj.tile([128, DK], F32R, tag="proj")
                nc.tensor.transpose(tr, vhT[:, 128 * j : 128 * (j + 1)], id_sb)
                nc.vector.tensor_copy(out=vhA[:, j, 0:DK], in_=tr)

            # ---- Q projection (pre-scaled) ----
            qc = []
            for i in range(NI):
                t = qpool.tile([128, SQ], F32R, tag="qc")
                nc.sync.dma_start(out=t, in_=qT_d[128 * i : 128 * (i + 1), :])
                qc.append(t)
            for t_i in range(NT):
                ts = slice(128 * t_i, 128 * (t_i + 1))
                qps = ps_proj.tile([128, SQ], F32, tag="proj")
                for i in range(NI):
                    nc.tensor.matmul(qps, wq_sb[i][:, ts], qc[i], start=(i == 0), stop=False)
                nc.tensor.matmul(qps, bq_sb[0:1, ts], ones_row, start=False, stop=True)
                nc.vector.tensor_copy(out=qhT[:, t_i, :], in_=qps)

            # ---- attention, head pairs ----
            for c in range(NT):
                cps = []
                for par in range(2):
                    half = slice(64 * par, 64 * par + 64)
                    ctx = ps_ctx.tile([DK + 1, SQ], F32, tag="ctx")
                    for j in range(NCH):
                        sc = ps_sc.tile([128, SQ], F32, tag="sc")
                        nc.tensor.matmul(
                            sc,
                            khT[half, 128 * j : 128 * (j + 1)],
                            qhT[half, c, :],
                            start=True,
                            stop=True,
                        )
                        ex = expp.tile([128, SQ], F32R, tag="expT")
                        nc.scalar.activation(out=ex, in_=sc, func=Exp)
                        nc.tensor.matmul(
                            ctx, vhA[:, j, :], ex, start=(j == 0), stop=(j == NCH - 1)
                        )
                    cps.append(ctx)
                R = small.tile([65, 2 * SQ], F32R, tag="R")
                with nc.allow_low_precision(reason="softmax denom tf32"):
                    nc.vector.reciprocal(out=R[64:65, 0:SQ], in_=cps[0][64:65, :])
                    nc.vector.reciprocal(out=R[64:65, SQ:], in_=cps[1][64:65, :])
                for par in range(2):
                    bc = ps_bc.tile([DK, SQ], F32, tag="bc")
                    nc.tensor.matmul(
                        bc, sel[64:65, :], R[64:65, SQ * par : SQ * (par + 1)],
                        start=True, stop=True,
                    )
                    bcs = small.tile([DK, SQ], F32R, tag="bcs")
                    nc.vector.tensor_copy(out=bcs, in_=bc)
                    if par == 0:
                        nc.vector.tensor_mul(ctxn[0:64, c, :], cps[0][0:DK, :], bcs)
                    else:
                        tmp = small.tile([DK, SQ], F32R, tag="tmp")
                        nc.vector.tensor_mul(tmp, cps[1][0:DK, :], bcs)
                        nc.sync.dma_start(out=ctxn[64:128, c, :], in_=tmp)

            # ---- O projection ----
            wo_sb = []
            for i in range(NI):
                w = wpool.tile([128, D], F32R, tag="bigw")
                nc.sync.dma_start(out=w, in_=wo_d[128 * i : 128 * (i + 1), :])
                wo_sb.append(w)
            for t_i in range(NT):
                ts = slice(128 * t_i, 128 * (t_i + 1))
                ops = ps_proj.tile([128, SQ], F32, tag="proj")
                for i in range(NI):
                    nc.tensor.matmul(ops, wo_sb[i][:, ts], ctxn[:, i, :], start=(i == 0), stop=False)
                nc.tensor.matmul(ops, bo_sb[0:1, ts], ones_row, start=False, stop=True)
                osb = small.tile([128, SQ], F32, tag="osb")
                nc.vector.tensor_copy(out=osb, in_=ops)
                nc.sync.dma_start(out=out_d[ts, :], in_=osb)

    nc.compile()
    return nc


def _get_nc():
    if "nc" not in _CACHE:
        _CACHE["nc"] = _build()
    return _CACHE["nc"]


def kernel(q, k, v, Wq, bq, Wk, bk, Wv, bv, Wo, bo, _trace=False):
    from concourse.bass_utils import run_bass_kernel_spmd

    q = np.ascontiguousarray(np.asarray(q, np.float32))
    k = np.ascontiguousarray(np.asarray(k, np.float32))
    v = np.ascontiguousarray(np.asarray(v, np.float32))
    common = {
        "WqTs": np.ascontiguousarray(np.asarray(Wq, np.float32).T * SCALE),
        "bqs": np.asarray(bq, np.float32).reshape(1, D) * SCALE,
        "WkT": np.ascontiguousarray(np.asarray(Wk, np.float32).T),
        "bk": np.asarray(bk, np.float32).reshape(1, DK),
        "WvT": np.ascontiguousarray(np.asarray(Wv, np.float32).T),
        "bv": np.asarray(bv, np.float32).reshape(1, DK),
        "WoT": np.ascontiguousarray(np.asarray(Wo, np.float32).T),
        "bo": np.asarray(bo, np.float32).reshape(1, D),
        "ident": np.eye(DK, dtype=np.float32),
        "ones": np.ones((1, SQ), np.float32),
    }
    kT = [np.ascontiguousarray(k[b].T) for b in range(B)]
    vT = [np.ascontiguousarray(v[b].T) for b in range(B)]
    in_maps = []
    for c in range(NC):
        b, r = c // 4, c % 4
        in_maps.append(
            dict(
                common,
                qT=np.ascontiguousarray(q[b, r * SQ : (r + 1) * SQ, :].T),
                kT=kT[b],
                vT=vT[b],
            )
        )

    nc = _get_nc()
    res = run_bass_kernel_spmd(nc, in_maps, core_ids=list(range(NC)), trace=_trace)
    _CACHE["last_result"] = res
    out = np.empty((B, S, D), np.float32)
    for c in range(NC):
        b, r = c // 4, c % 4
        out[b, r * SQ : (r + 1) * SQ, :] = res.results[c]["outT"].T
    return out


# revision 5
# speedup vs baseline: 1.5901x; 1.5901x over previous
"""Trainium2 Bass kernel for CPU-optimized Multi-Query Attention.

Full inputs in, full output out. Sharding: data-parallel over batch (2) x
sequence-parallel over queries (4) = 8 cores, no collectives. Each core:
  - projects its 512-query slice (q) plus the full K/V of its batch,
  - runs 16-head MQA attention in transposed layouts (scoresT = [keys, queries])
    so softmax sums ride as an extra ones-column in the ctx matmul,
  - o-projects and writes outT [1024, 512]; host transposes/concats.
Matmul operands are fp16 (1 cycle/column on the PE, fp32 PSUM accumulate);
heads are processed in pairs so exp runs on [128, 1024] tiles and the
ctx matmul covers both heads of a pair in one N=1024 instruction.
"""
import numpy as np

B, S, D, H, DK = 2, 2048, 1024, 16, 64
SQ = S // 4          # queries per core
NC = 8               # cores
SCALE = DK ** -0.5
NCH = S // 128       # 16 key chunks
NI = D // 128        # 8 contraction chunks for projections
NT = D // 128        # 8 output tiles / head pairs

_CACHE = {}


def _build():
    import concourse.bacc as bacc
    import concourse.bass as bass
    import concourse.tile as tile
    from concourse import mybir

    F32 = mybir.dt.float32
    F16 = mybir.dt.float16
    Exp = mybir.ActivationFunctionType.Exp

    nc = bacc.Bacc("TRN2", target_bir_lowering=False, debug=False, num_devices=NC)

    qT_d = nc.dram_tensor("qT", [D, SQ], F16, kind="ExternalInput").ap()
    kT_d = nc.dram_tensor("kT", [D, S], F16, kind="ExternalInput").ap()
    vT_d = nc.dram_tensor("vT", [D, S], F16, kind="ExternalInput").ap()
    wq_d = nc.dram_tensor("WqTs", [D, D], F16, kind="ExternalInput").ap()
    bq_d = nc.dram_tensor("bqs", [1, D], F16, kind="ExternalInput").ap()
    wk_d = nc.dram_tensor("WkT", [D, DK], F16, kind="ExternalInput").ap()
    bk_d = nc.dram_tensor("bk", [1, DK], F16, kind="ExternalInput").ap()
    wv_d = nc.dram_tensor("WvT", [D, DK], F16, kind="ExternalInput").ap()
    bv_d = nc.dram_tensor("bv", [1, DK], F16, kind="ExternalInput").ap()
    wo_d = nc.dram_tensor("WoT", [D, D], F16, kind="ExternalInput").ap()
    bo_d = nc.dram_tensor("bo", [1, D], F16, kind="ExternalInput").ap()
    id_d = nc.dram_tensor("ident", [DK, DK], F16, kind="ExternalInput").ap()
    ones_h = nc.dram_tensor("ones", [1, SQ], F16, kind="ExternalInput")
    ones_d = ones_h.ap()
    out_d = nc.dram_tensor("outT", [D, SQ], F32, kind="ExternalOutput").ap()

    with tile.TileContext(nc) as tc:
        with (
            tc.tile_pool(name="single", bufs=1) as single,
            tc.tile_pool(name="wpool", bufs=8) as wpool,
            tc.tile_pool(name="stream", bufs=10) as stream,
            tc.tile_pool(name="qpool", bufs=8) as qpool,
            tc.tile_pool(name="expp", bufs=6) as expp,
            tc.tile_pool(name="small", bufs=2) as small,
            # PSUM: pool A 2x2-bank slots (scores pairs / projections / bcast),
            #       pool B 2x2-bank slots (ctx pair accumulators / transposes)
            tc.tile_pool(name="ps_a", bufs=2, space="PSUM") as ps_a,
            tc.tile_pool(name="ps_b", bufs=2, space="PSUM") as ps_b,
        ):
            # ---- persistent SBUF ----
            khT = single.tile([128, S], F16)           # kh^T duplicated halves
            vhT = single.tile([DK, S], F16)
            vhA = single.tile([128, NCH, DK + 1], F16)  # [vh | ones] per chunk
            qhT = single.tile([128, NT, SQ], F16)
            ctxn = single.tile([128, NT, SQ], F16)
            wk_sb = single.tile([128, NI, DK], F16)
            wv_sb = single.tile([128, NI, DK], F16)
            bk_sb = single.tile([1, DK], F16)
            bv_sb = single.tile([1, DK], F16)
            bq_sb = single.tile([1, D], F16)
            bo_sb = single.tile([1, D], F16)
            id_sb = single.tile([DK, DK], F16)
            ones_row = single.tile([1, SQ], F16)
            sel = single.tile([65, DK], F16)

            # ---- constant / weight loads ----
            nc.sync.dma_start(out=ones_row, in_=ones_d)
            nc.sync.dma_start(out=sel[64:65, :], in_=ones_d[0:1, 0:DK])
            nc.sync.dma_start(out=id_sb, in_=id_d)
            nc.sync.dma_start(out=bk_sb, in_=bk_d)
            nc.sync.dma_start(out=bv_sb, in_=bv_d)
            nc.sync.dma_start(out=bq_sb, in_=bq_d)
            nc.sync.dma_start(out=bo_sb, in_=bo_d)
            nc.sync.dma_start(out=wk_sb, in_=wk_d.rearrange("(c p) d -> p c d", p=128))
            nc.sync.dma_start(out=wv_sb, in_=wv_d.rearrange("(c p) d -> p c d", p=128))
            nc.sync.dma_start(
                out=vhA[:, :, DK : DK + 1],
                in_=bass.AP(tensor=ones_h, offset=0, ap=[[0, 128], [0, NCH], [1, 1]]),
            )
            wq_sb = []
            for i in range(NI):
                w = wpool.tile([128, D], F16, tag="bigw")
                nc.sync.dma_start(out=w, in_=wq_d[128 * i : 128 * (i + 1), :])
                wq_sb.append(w)

            # ---- K/V projections (full batch sequence) ----
            for j in range(4):
                js = slice(512 * j, 512 * (j + 1))
                kc, vc = [], []
                for i in range(NI):
                    t = stream.tile([128, 512], F16, tag="stream")
                    nc.sync.dma_start(out=t, in_=kT_d[128 * i : 128 * (i + 1), js])
                    kc.append(t)
                for i in range(NI):
                    t = stream.tile([128, 512], F16, tag="stream")
                    nc.sync.dma_start(out=t, in_=vT_d[128 * i : 128 * (i + 1), js])
                    vc.append(t)
                kps = ps_a.tile([DK, 512], F32, tag="a")
                for i in range(NI):
                    nc.tensor.matmul(kps, wk_sb[:, i, :], kc[i], start=(i == 0), stop=False)
                nc.tensor.matmul(kps, bk_sb, ones_row, start=False, stop=True)
                nc.vector.tensor_copy(out=khT[0:DK, js], in_=kps)
                vps = ps_a.tile([DK, 512], F32, tag="a")
                for i in range(NI):
                    nc.tensor.matmul(vps, wv_sb[:, i, :], vc[i], start=(i == 0), stop=False)
                nc.tensor.matmul(vps, bv_sb, ones_row, start=False, stop=True)
                nc.vector.tensor_copy(out=vhT[:, js], in_=vps)
                # duplicate khT rows into partitions 64..127 for row-packed matmuls
                nc.sync.dma_start(out=khT[64:128, js], in_=khT[0:DK, js])

            # vh = vhT^T via PE transpose, into [vh | ones] tiles
            for j in range(NCH):
                tr = ps_b.tile([128, DK], F16, tag="b")
                nc.tensor.transpose(tr, vhT[:, 128 * j : 128 * (j + 1)], id_sb)
                nc.vector.tensor_copy(out=vhA[:, j, 0:DK], in_=tr)

            qc = []
            for i in range(NI):
                t = qpool.tile([128, SQ], F16, tag="qc")
                nc.sync.dma_start(out=t, in_=qT_d[128 * i : 128 * (i + 1), :])
                qc.append(t)

            def q_proj(t_i):
                ts = slice(128 * t_i, 128 * (t_i + 1))
                qps = ps_a.tile([128, SQ], F32, tag="a")
                for i in range(NI):
                    nc.tensor.matmul(qps, wq_sb[i][:, ts], qc[i], start=(i == 0), stop=False)
                nc.tensor.matmul(qps, bq_sb[0:1, ts], ones_row, start=False, stop=True)
                nc.vector.tensor_copy(out=qhT[:, t_i, :], in_=qps)

            # ---- attention: groups of 2 head-pairs, chunk-major ----
            lo, hi = slice(0, 64), slice(64, 128)
            for g in range(NT // 2):
                p0, p1 = 2 * g, 2 * g + 1
                q_proj(p0)
                q_proj(p1)
                ctx0 = ps_b.tile([DK + 1, 2 * SQ], F32, tag="b")
                ctx1 = ps_b.tile([DK + 1, 2 * SQ], F32, tag="b")
                for j in range(NCH):
                    jsl = slice(128 * j, 128 * (j + 1))
                    sc0 = ps_a.tile([128, 2 * SQ], F32, tag="a")
                    sc1 = ps_a.tile([128, 2 * SQ], F32, tag="a")
                    # shared-lhsT score matmuls; lo/hi halves run on distinct
                    # PE row groups (concurrent)
                    nc.tensor.matmul(sc0[:, 0:SQ], khT[lo, jsl], qhT[lo, p0, :], start=True, stop=True)
                    nc.tensor.matmul(sc1[:, 0:SQ], khT[lo, jsl], qhT[lo, p1, :], start=True, stop=True)
                    nc.tensor.matmul(sc0[:, SQ:], khT[hi, jsl], qhT[hi, p0, :], start=True, stop=True)
                    nc.tensor.matmul(sc1[:, SQ:], khT[hi, jsl], qhT[hi, p1, :], start=True, stop=True)
                    ex0 = expp.tile([128, 2 * SQ], F16, tag="expT")
                    ex1 = expp.tile([128, 2 * SQ], F16, tag="expT")
                    nc.scalar.activation(out=ex0, in_=sc0, func=Exp)
                    nc.scalar.activation(out=ex1, in_=sc1, func=Exp)
                    st, sp = (j == 0), (j == NCH - 1)
                    nc.tensor.matmul(ctx0[:, 0:SQ], vhA[:, j, :], ex0[:, 0:SQ], start=st, stop=sp)
                    nc.tensor.matmul(ctx0[:, SQ:], vhA[:, j, :], ex0[:, SQ:], start=st, stop=sp)
                    nc.tensor.matmul(ctx1[:, 0:SQ], vhA[:, j, :], ex1[:, 0:SQ], start=st, stop=sp)
                    nc.tensor.matmul(ctx1[:, SQ:], vhA[:, j, :], ex1[:, SQ:], start=st, stop=sp)
                for p, ctx in ((p0, ctx0), (p1, ctx1)):
                    R = small.tile([65, 2 * SQ], F16, tag="R")
                    with nc.allow_low_precision(reason="softmax denom"):
                        nc.vector.reciprocal(out=R[64:65, :], in_=ctx[64:65, :])
                    bc = ps_a.tile([DK, 2 * SQ], F32, tag="a")
                    nc.tensor.matmul(bc[:, 0:SQ], sel[64:65, :], R[64:65, 0:SQ], start=True, stop=True)
                    nc.tensor.matmul(bc[:, SQ:], sel[64:65, :], R[64:65, SQ:], start=True, stop=True)
                    bcs = small.tile([DK, 2 * SQ], F16, tag="bcs")
                    nc.vector.tensor_copy(out=bcs, in_=bc)
                    nc.vector.tensor_mul(ctxn[0:64, p, :], ctx[0:DK, 0:SQ], bcs[:, 0:SQ])
                    tmp = small.tile([DK, SQ], F16, tag="tmp")
                    nc.vector.tensor_mul(tmp, ctx[0:DK, SQ:], bcs[:, SQ:])
                    nc.sync.dma_start(out=ctxn[64:128, p, :], in_=tmp)

            # ---- O projection ----
            wo_sb = []
            for i in range(NI):
                w = wpool.tile([128, D], F16, tag="bigw")
                nc.sync.dma_start(out=w, in_=wo_d[128 * i : 128 * (i + 1), :])
                wo_sb.append(w)
            for t_i in range(NT):
                ts = slice(128 * t_i, 128 * (t_i + 1))
                ops = ps_a.tile([128, SQ], F32, tag="a")
                for i in range(NI):
                    nc.tensor.matmul(ops, wo_sb[i][:, ts], ctxn[:, i, :], start=(i == 0), stop=False)
                nc.tensor.matmul(ops, bo_sb[0:1, ts], ones_row, start=False, stop=True)
                osb = small.tile([128, SQ], F32, tag="osb")
                nc.vector.tensor_copy(out=osb, in_=ops)
                nc.sync.dma_start(out=out_d[ts, :], in_=osb)

    nc.compile()
    return nc


def _get_nc():
    if "nc" not in _CACHE:
        _CACHE["nc"] = _build()
    return _CACHE["nc"]


def kernel(q, k, v, Wq, bq, Wk, bk, Wv, bv, Wo, bo, _trace=False):
    from concourse.bass_utils import run_bass_kernel_spmd

    f16 = np.float16
    q = np.asarray(q, np.float32)
    k = np.asarray(k, np.float32)
    v = np.asarray(v, np.float32)
    common = {
        "WqTs": np.ascontiguousarray((np.asarray(Wq, np.float32).T * SCALE).astype(f16)),
        "bqs": (np.asarray(bq, np.float32).reshape(1, D) * SCALE).astype(f16),
        "WkT": np.ascontiguousarray(np.asarray(Wk, np.float32).T.astype(f16)),
        "bk": np.asarray(bk, f16).reshape(1, DK),
        "WvT": np.ascontiguousarray(np.asarray(Wv, np.float32).T.astype(f16)),
        "bv": np.asarray(bv, f16).reshape(1, DK),
        "WoT": np.ascontiguousarray(np.asarray(Wo, np.float32).T.astype(f16)),
        "bo": np.asarray(bo, f16).reshape(1, D),
        "ident": np.eye(DK, dtype=f16),
        "ones": np.ones((1, SQ), f16),
    }
    kT = [np.ascontiguousarray(k[b].T.astype(f16)) for b in range(B)]
    vT = [np.ascontiguousarray(v[b].T.astype(f16)) for b in range(B)]
    in_maps = []
    for c in range(NC):
        b, r = c // 4, c % 4
        in_maps.append(
            dict(
                common,
                qT=np.ascontiguousarray(q[b, r * SQ : (r + 1) * SQ, :].T.astype(f16)),
                kT=kT[b],
                vT=vT[b],
            )
        )

    nc = _get_nc()
    res = run_bass_kernel_spmd(nc, in_maps, core_ids=list(range(NC)), trace=_trace)
    _CACHE["last_result"] = res
    out = np.empty((B, S, D), np.float32)
    for c in range(NC):
        b, r = c // 4, c % 4
        out[b, r * SQ : (r + 1) * SQ, :] = res.results[c]["outT"].T
    return out


# revision 12
# speedup vs baseline: 1.8008x; 1.1325x over previous
"""Trainium2 Bass kernel for CPU-optimized Multi-Query Attention.

Full inputs in, full output out. Sharding: data-parallel over batch (2) x
sequence-parallel over queries (4) = 8 cores, no collectives. Each core:
  - projects its 512-query slice (q) plus the full K/V of its batch,
  - runs 16-head MQA attention in transposed layouts (scoresT = [keys, queries])
    so softmax sums ride as an extra ones-column in the ctx matmul,
  - o-projects and writes outT [1024, 512]; host transposes/concats.
Matmul operands are fp16 (1 cycle/column on the PE, fp32 PSUM accumulate);
head pairs share score/ctx stationary operands, exp runs on [128, 1024]
PSUM pair-tiles, softmax normalization runs SBUF-only (reciprocal +
stride-0-broadcast DMA) so it never blocks the next group's PSUM.
Inputs are host-pre-tiled so DMAs move >=8KB per partition line; group 0's
attention is pipelined into the K/V projection blocks.
"""
import numpy as np

B, S, D, H, DK = 2, 2048, 1024, 16, 64
SQ = S // 4          # queries per core
NC = 8               # cores
SCALE = DK ** -0.5
NCH = S // 128       # 16 key chunks
NI = D // 128        # 8 contraction chunks for projections
NT = D // 128        # 8 output tiles / head pairs

_CACHE = {}


def _build():
    import concourse.bacc as bacc
    import concourse.bass as bass
    import concourse.tile as tile
    from concourse import mybir

    F32 = mybir.dt.float32
    F16 = mybir.dt.float16
    Exp = mybir.ActivationFunctionType.Exp

    nc = bacc.Bacc("TRN2", target_bir_lowering=False, debug=False, num_devices=NC)

    qT_d = nc.dram_tensor("qT", [128, NI, SQ], F16, kind="ExternalInput").ap()
    kT_d = nc.dram_tensor("kT", [4, 128, NI, 512], F16, kind="ExternalInput").ap()
    vT_d = nc.dram_tensor("vT", [4, 128, NI, 512], F16, kind="ExternalInput").ap()
    wq_d = nc.dram_tensor("WqTs", [128, NI, D], F16, kind="ExternalInput").ap()
    bq_d = nc.dram_tensor("bqs", [1, D], F16, kind="ExternalInput").ap()
    wk_d = nc.dram_tensor("WkT", [128, NI, DK], F16, kind="ExternalInput").ap()
    bk_d = nc.dram_tensor("bk", [1, DK], F16, kind="ExternalInput").ap()
    wv_d = nc.dram_tensor("WvT", [128, NI, DK], F16, kind="ExternalInput").ap()
    bv_d = nc.dram_tensor("bv", [1, DK], F16, kind="ExternalInput").ap()
    wo_d = nc.dram_tensor("WoT", [128, NI, D], F16, kind="ExternalInput").ap()
    bo_d = nc.dram_tensor("bo", [1, D], F16, kind="ExternalInput").ap()
    id_d = nc.dram_tensor("ident", [DK, DK], F16, kind="ExternalInput").ap()
    ones_h = nc.dram_tensor("ones", [1, SQ], F16, kind="ExternalInput")
    ones_d = ones_h.ap()
    out_d = nc.dram_tensor("outT", [D, SQ], F32, kind="ExternalOutput").ap()

    with tile.TileContext(nc) as tc:
        with (
            tc.tile_pool(name="single", bufs=1) as single,
            tc.tile_pool(name="stream", bufs=4) as stream,
            tc.tile_pool(name="expp", bufs=8) as expp,
            tc.tile_pool(name="small", bufs=2) as small,
            tc.tile_pool(name="drp", bufs=2, space="DRAM") as drp,
            # PSUM: pool A 2x2-bank slots (score pairs / projections),
            #       pool B 2x2-bank slots (ctx pair accumulators / transposes)
            tc.tile_pool(name="ps_a", bufs=2, space="PSUM") as ps_a,
            tc.tile_pool(name="ps_b", bufs=2, space="PSUM") as ps_b,
        ):
            # ---- persistent SBUF ----
            khT = single.tile([128, S], F16)           # kh^T duplicated halves
            vhT = single.tile([DK, S], F16)
            vhA = single.tile([128, NCH, DK + 1], F16)  # [vh | ones] per chunk
            qhT = single.tile([128, NT, SQ], F16)
            ctxn = single.tile([128, NT, SQ], F16)
            wq_sb = single.tile([128, NI, D], F16)
            wo_sb = single.tile([128, NI, D], F16)
            qc = single.tile([128, NI, SQ], F16)
            wk_sb = single.tile([128, NI, DK], F16)
            wv_sb = single.tile([128, NI, DK], F16)
            bk_sb = single.tile([1, DK], F16)
            bv_sb = single.tile([1, DK], F16)
            bq_sb = single.tile([1, D], F16)
            bo_sb = single.tile([1, D], F16)
            id_sb = single.tile([DK, DK], F16)
            ones_row = single.tile([1, SQ], F16)

            # ---- constants, then K/V block 0, then q-side weights ----
            nc.sync.dma_start(out=ones_row, in_=ones_d)
            nc.sync.dma_start(out=id_sb, in_=id_d)
            nc.sync.dma_start(out=bk_sb, in_=bk_d)
            nc.sync.dma_start(out=bv_sb, in_=bv_d)
            nc.sync.dma_start(out=bq_sb, in_=bq_d)
            nc.sync.dma_start(out=bo_sb, in_=bo_d)
            nc.sync.dma_start(out=wk_sb, in_=wk_d)
            nc.sync.dma_start(out=wv_sb, in_=wv_d)
            nc.sync.dma_start(
                out=vhA[:, :, DK : DK + 1],
                in_=bass.AP(tensor=ones_h, offset=0, ap=[[0, 128], [0, NCH], [1, 1]]),
            )

            def kv_block(j):
                js = slice(512 * j, 512 * (j + 1))
                kc = stream.tile([128, NI, 512], F16, tag="stream")
                nc.sync.dma_start(out=kc, in_=kT_d[j])
                vc = stream.tile([128, NI, 512], F16, tag="stream")
                nc.sync.dma_start(out=vc, in_=vT_d[j])
                kps = ps_a.tile([DK, 512], F32, tag="a")
                for i in range(NI):
                    nc.tensor.matmul(kps, wk_sb[:, i, :], kc[:, i, :], start=(i == 0), stop=False)
                nc.tensor.matmul(kps, bk_sb, ones_row, start=False, stop=True)
                nc.vector.tensor_copy(out=khT[0:DK, js], in_=kps)
                vps = ps_a.tile([DK, 512], F32, tag="a")
                for i in range(NI):
                    nc.tensor.matmul(vps, wv_sb[:, i, :], vc[:, i, :], start=(i == 0), stop=False)
                nc.tensor.matmul(vps, bv_sb, ones_row, start=False, stop=True)
                nc.vector.tensor_copy(out=vhT[:, js], in_=vps)
                # duplicate khT rows into partitions 64..127 for row-packed matmuls
                nc.sync.dma_start(out=khT[64:128, js], in_=khT[0:DK, js])
                # vh = vhT^T via PE transpose, into [vh | ones] tiles
                for jj in range(4 * j, 4 * j + 4):
                    tr = ps_a.tile([128, DK], F16, tag="a")
                    nc.tensor.transpose(tr, vhT[:, 128 * jj : 128 * (jj + 1)], id_sb)
                    nc.vector.tensor_copy(out=vhA[:, jj, 0:DK], in_=tr)

            kv_block(0)
            nc.sync.dma_start(out=qc, in_=qT_d)
            nc.sync.dma_start(out=wq_sb, in_=wq_d)

            def q_proj(t_i):
                ts = slice(128 * t_i, 128 * (t_i + 1))
                qps = ps_a.tile([128, SQ], F32, tag="a")
                for i in range(NI):
                    nc.tensor.matmul(qps, wq_sb[:, i, ts], qc[:, i, :], start=(i == 0), stop=False)
                nc.tensor.matmul(qps, bq_sb[0:1, ts], ones_row, start=False, stop=True)
                nc.vector.tensor_copy(out=qhT[:, t_i, :], in_=qps)

            for t_i in range(NT):
                q_proj(t_i)

            lo, hi = slice(0, 64), slice(64, 128)

            def att_chunks(p0, p1, ctx0, ctx1, chunks):
                for j in chunks:
                    jsl = slice(128 * j, 128 * (j + 1))
                    sc0 = ps_a.tile([128, 2 * SQ], F32, tag="a")
                    sc1 = ps_a.tile([128, 2 * SQ], F32, tag="a")
                    # shared-lhsT score matmuls; lo/hi halves run on distinct
                    # PE row groups (concurrent)
                    nc.tensor.matmul(sc0[:, 0:SQ], khT[lo, jsl], qhT[lo, p0, :], start=True, stop=True)
                    nc.tensor.matmul(sc1[:, 0:SQ], khT[lo, jsl], qhT[lo, p1, :], start=True, stop=True)
                    nc.tensor.matmul(sc0[:, SQ:], khT[hi, jsl], qhT[hi, p0, :], start=True, stop=True)
                    nc.tensor.matmul(sc1[:, SQ:], khT[hi, jsl], qhT[hi, p1, :], start=True, stop=True)
                    ex0 = expp.tile([128, 2 * SQ], F16, tag="expT")
                    ex1 = expp.tile([128, 2 * SQ], F16, tag="expT")
                    nc.scalar.activation(out=ex0, in_=sc0, func=Exp)
                    nc.scalar.activation(out=ex1, in_=sc1, func=Exp)
                    st, sp = (j == 0), (j == NCH - 1)
                    nc.tensor.matmul(ctx0[:, 0:SQ], vhA[:, j, :], ex0[:, 0:SQ], start=st, stop=sp)
                    nc.tensor.matmul(ctx0[:, SQ:], vhA[:, j, :], ex0[:, SQ:], start=st, stop=sp)
                    nc.tensor.matmul(ctx1[:, 0:SQ], vhA[:, j, :], ex1[:, 0:SQ], start=st, stop=sp)
                    nc.tensor.matmul(ctx1[:, SQ:], vhA[:, j, :], ex1[:, SQ:], start=st, stop=sp)

            def norm(p, ctx):
                # copy ctx out of PSUM first (releases the accumulator), then
                # normalize SBUF-only: reciprocal + stride-0 broadcast DMA
                cs = small.tile([DK, 2 * SQ], F16, tag="cs")
                nc.vector.tensor_copy(out=cs, in_=ctx[0:DK, :])
                R = small.tile([65, 2 * SQ], F32, tag="R")
                nc.vector.tensor_copy(out=R[64:65, :], in_=ctx[64:65, :])
                R2 = small.tile([65, 2 * SQ], F32, tag="R2")
                with nc.allow_low_precision(reason="softmax denom"):
                    nc.vector.reciprocal(out=R2[64:65, :], in_=R[64:65, :])
                dr = drp.tile([1, 2 * SQ], F32, tag="dr")
                nc.sync.dma_start(out=dr, in_=R2[64:65, :])
                drow = dr[0:1, :]
                bcast = bass.AP(
                    tensor=drow.tensor, offset=drow.offset, ap=[[0, DK], [1, 2 * SQ]]
                )
                bcs = small.tile([DK, 2 * SQ], F32, tag="bcs")
                nc.sync.dma_start(out=bcs, in_=bcast)
                nc.vector.tensor_mul(ctxn[0:64, p, :], cs[:, 0:SQ], bcs[:, 0:SQ])
                tmp = small.tile([DK, SQ], F16, tag="tmp")
                nc.vector.tensor_mul(tmp, cs[:, SQ:], bcs[:, SQ:])
                nc.sync.dma_start(out=ctxn[64:128, p, :], in_=tmp)

            # group 0 pipelined into the remaining K/V blocks
            ctx0 = ps_b.tile([DK + 1, 2 * SQ], F32, tag="b")
            ctx1 = ps_b.tile([DK + 1, 2 * SQ], F32, tag="b")
            att_chunks(0, 1, ctx0, ctx1, range(0, 4))
            for b in range(1, 4):
                kv_block(b)
                att_chunks(0, 1, ctx0, ctx1, range(4 * b, 4 * b + 4))
            norm(0, ctx0)
            norm(1, ctx1)

            for g in range(1, NT // 2):
                p0, p1 = 2 * g, 2 * g + 1
                if g == 3:
                    nc.sync.dma_start(out=wo_sb, in_=wo_d)
                ctx0 = ps_b.tile([DK + 1, 2 * SQ], F32, tag="b")
                ctx1 = ps_b.tile([DK + 1, 2 * SQ], F32, tag="b")
                att_chunks(p0, p1, ctx0, ctx1, range(NCH))
                norm(p0, ctx0)
                norm(p1, ctx1)

            # ---- O projection ----
            for t_i in range(NT):
                ts = slice(128 * t_i, 128 * (t_i + 1))
                ops = ps_a.tile([128, SQ], F32, tag="a")
                for i in range(NI):
                    nc.tensor.matmul(ops, wo_sb[:, i, ts], ctxn[:, i, :], start=(i == 0), stop=False)
                nc.tensor.matmul(ops, bo_sb[0:1, ts], ones_row, start=False, stop=True)
                osb = small.tile([128, SQ], F32, tag="osb")
                nc.vector.tensor_copy(out=osb, in_=ops)
                nc.sync.dma_start(out=out_d[ts, :], in_=osb)

    nc.compile()
    return nc


def _get_nc():
    if "nc" not in _CACHE:
        _CACHE["nc"] = _build()
    return _CACHE["nc"]


def _tile_pi(a, p=128):
    """[P*NI, F] -> [P, NI, F] with chunk i = rows [128i, 128(i+1))."""
    n = a.shape[0] // p
    return np.ascontiguousarray(a.reshape(n, p, a.shape[1]).transpose(1, 0, 2))


def kernel(q, k, v, Wq, bq, Wk, bk, Wv, bv, Wo, bo, _trace=False):
    from concourse.bass_utils import run_bass_kernel_spmd

    f16 = np.float16
    q = np.asarray(q, np.float32)
    k = np.asarray(k, np.float32)
    v = np.asarray(v, np.float32)
    common = {
        "WqTs": _tile_pi((np.asarray(Wq, np.float32).T * SCALE).astype(f16)),
        "bqs": (np.asarray(bq, np.float32).reshape(1, D) * SCALE).astype(f16),
        "WkT": _tile_pi(np.asarray(Wk, np.float32).T.astype(f16)),
        "bk": np.asarray(bk, f16).reshape(1, DK),
        "WvT": _tile_pi(np.asarray(Wv, np.float32).T.astype(f16)),
        "bv": np.asarray(bv, f16).reshape(1, DK),
        "WoT": _tile_pi(np.asarray(Wo, np.float32).T.astype(f16)),
        "bo": np.asarray(bo, f16).reshape(1, D),
        "ident": np.eye(DK, dtype=f16),
        "ones": np.ones((1, SQ), f16),
    }

    def _tile_kv(x):  # [D, S] -> [4, 128, NI, 512]
        return np.ascontiguousarray(
            x.reshape(NI, 128, 4, 512).transpose(2, 1, 0, 3)
        )

    kT = [_tile_kv(k[b].T.astype(f16)) for b in range(B)]
    vT = [_tile_kv(v[b].T.astype(f16)) for b in range(B)]
    in_maps = []
    for c in range(NC):
        b, r = c // 4, c % 4
        in_maps.append(
            dict(
                common,
                qT=_tile_pi(q[b, r * SQ : (r + 1) * SQ, :].T.astype(f16)),
                kT=kT[b],
                vT=vT[b],
            )
        )

    nc = _get_nc()
    res = run_bass_kernel_spmd(nc, in_maps, core_ids=list(range(NC)), trace=_trace)
    _CACHE["last_result"] = res
    out = np.empty((B, S, D), np.float32)
    for c in range(NC):
        b, r = c // 4, c % 4
        out[b, r * SQ : (r + 1) * SQ, :] = res.results[c]["outT"].T
    return out
